# revision 34
# baseline (speedup 1.0000x reference)
"""Two-layer GAT (4-head then 1-head) on 8 NeuronCores.

Sharding: nodes are partitioned across the 8 cores by dst-ownership after a
degree-balancing permutation (snake-deal of degree-sorted nodes into 392
windows of 128 nodes, max in-degree per window <= 2048 = C*128 with C=16).
Each core processes all edges whose dst it owns.  Per-dst-window the segment
softmax + weighted aggregation run as one-hot-selection matmuls on the tensor
engine.

Key kernel techniques vs the naive formulation:
  - per-edge attention exp() is computed on the small [slot, w2*c*h] tile,
    then broadcast across the 64 features by log2-doubling copies whose
    inner runs are contiguous (layout (w2, c, a, h) with (w2, c) outermost),
    split between the DVE and ACT engines;
  - matmul rhs slices [128, 260] are fully contiguous per (window, chunk);
  - windows are processed in pairs so elementwise instruction overheads are
    amortized;
  - all big DMAs are partition-major single-run transfers batched over
    multiple windows;
  - h1 is re-transposed through the PE and layer-1 bias+relu run as a single
    fused two-op tensor_scalar on the DVE; the ACT engine only ever runs Exp
    and Copy so activation-table reloads are minimized.

Three SPMD launches:
  K1: h|el|er = x @ [W0^T | vl0^T | vr0^T]   (node-sharded, bf16)
  K2: L0 edge phase (attention + aggregation) + relu + L1 node matmul
  K3: L1 edge phase -> output

Between launches the host performs pure index gathers / dtype casts of
device-computed tables; all floating-point math runs on device.
"""
import os
import sys
import types

sys.path.insert(0, "/opt/trn_rl_repo")

import numpy as np

import concourse.bass as bass
import concourse.tile as tile
from concourse import mybir
from concourse.bass_utils import run_bass_kernel_spmd
from concourse.vector_clock import ScopedClock

# ---------------------------------------------------------------- constants
N_NODES = 50000
IN_F = 256
HID = 64
HEADS = 4
OUT_F = 64
NEG_SLOPE = 0.2

NC_CORES = 8
P = 128
W_PER_CORE = 49
OWN = W_PER_CORE * P            # 6272 nodes per core
PADN = NC_CORES * OWN           # 50176
C = 16                          # edge chunks per window (cap C*128 = 2048)
W2 = 50                         # windows padded to even count for pairing
NPAIR = W2 // 2                 # 25

F32 = mybir.dt.float32
BF = mybir.dt.bfloat16
F8 = mybir.dt.float8e4

NB2 = 2 * C                     # 32 (w2, c) blocks per pair, layer 0/1
HE2 = NB2 * 65 * HEADS          # 8320  he cols per pair (a=0 ones | a=1..64)
MW2 = 2 * C * 8                 # 256   meta cols per pair
PW = HE2 + MW2                  # 8576

GE2 = NB2 * 65                  # 2080
MW3 = 2 * 2 * C                 # 64
PW3 = GE2 + MW3                 # 2144

G = OUT_F + 2                   # 66

EXEC_TIMES_NS = {}              # filled when GAT_PROFILE=1


# ------------------------------------------------------------- tile patches
def _patch_tile():
    """This container's walrus rejects instructions with >1 sem wait
    ("Too many sync wait commands").  After Tile lowering, move excess waits
    onto same-engine no-ops inserted before the offending instruction."""
    if getattr(_patch_tile, "done", False):
        return
    _patch_tile.done = True

    MAX_WAITS = 1

    def _split_all_waits(nc):
        for bb in nc.main_func.blocks:
            insts = bb.instructions
            i = 0
            while i < len(insts):
                inst = insts[i]
                si = getattr(inst, "sync_info", None)
                if si is None or len(si.on_wait) <= MAX_WAITS:
                    i += 1
                    continue
                waits = list(si.on_wait)
                si.on_wait[:] = waits[:MAX_WAITS]
                extra = waits[MAX_WAITS:]
                nops = []
                for j in range(0, len(extra), MAX_WAITS):
                    nop = mybir.InstNoOp(
                        name=f"I-waitsplit-{nc.next_id()}",
                        ins=[],
                        outs=[],
                        engine=inst.engine,
                    )
                    nop.sync_info = mybir.SyncInfo(
                        on_wait=extra[j : j + MAX_WAITS], on_update=[]
                    )
                    nc.register_instruction(nop, overwrite=True)
                    nops.append(nop)
                insts[i:i] = nops
                i += len(nops) + 1

    def _drain_and_barrier(self, tick_clock, wait_clock):
        drain_inst = self.nc.sync.drain()
        wait_clock.add_sem_waits(
            drain_inst.ins, ScopedClock({None: tick_clock.global_clock})
        )
        self.nc.all_engine_barrier()
        assert self.sems is not None
        popped = self.nc._tile_sem_poison_stack.pop()
        assert popped is self._sem_poison
        self.nc.clear_and_free_semaphores(list(self.sems.allocated().values()))
        self.nc.all_engine_barrier()
        _split_all_waits(self.nc)

    tile.TileContext._drain_and_barrier = _drain_and_barrier


def _install_ntff_hook():
    """Enable run_bass_kernel_spmd(trace=True) under axon: register the NTFF
    profile hook that the boot script skips when antenv.axon_hooks is absent."""
    if getattr(_install_ntff_hook, "done", False):
        return
    _install_ntff_hook.done = True
    try:
        mod = types.ModuleType("antenv.axon_hooks")
        _state = {}

        def set_axon_ntff_profile_hook(h):
            _state["h"] = h

        def get_axon_ntff_profile_hook():
            return _state.get("h")

        mod.set_axon_ntff_profile_hook = set_axon_ntff_profile_hook
        mod.get_axon_ntff_profile_hook = get_axon_ntff_profile_hook
        sys.modules["antenv.axon_hooks"] = mod
        import antenv

        antenv.axon_hooks = mod
        from trn_agent_boot.trn_boot import _ntff_profile_via_ctypes

        hook = _ntff_profile_via_ctypes("/opt/axon/libaxon_pjrt.so")
        if hook is not None:
            set_axon_ntff_profile_hook(hook)
    except Exception:
        pass


# ------------------------------------------------------------- kernel builders
def build_k1():
    """h|el|er tables for this core's 6272 nodes (bf16 matmul + bf16 out)."""
    nc = bass.Bass()
    DE = IN_F + 2 * HEADS                     # 264
    xT_own = nc.dram_tensor("xT_own", [IN_F, OWN], BF, kind="ExternalInput")
    w0te = nc.dram_tensor("w0te", [IN_F, DE], BF, kind="ExternalInput")
    htab_h = nc.dram_tensor("htab_h", [P, W_PER_CORE, IN_F], BF, kind="ExternalOutput")
    htab_e = nc.dram_tensor("htab_e", [P, W_PER_CORE, 8], BF, kind="ExternalOutput")

    NB = 7                                    # output batch (49 = 7*7)
    NXS = 7                                   # xk loaded in 7-window slabs
    with tile.TileContext(nc) as tc:
        with (
            tc.tile_pool(name="const", bufs=1) as constp,
            tc.tile_pool(name="sbuf", bufs=2) as pool,
            tc.tile_pool(name="psum", bufs=4, space="PSUM") as psum,
        ):
            wt = constp.tile([P, 2, DE], BF)
            nc.sync.dma_start(wt[:, 0, :], w0te[0:P, :])
            nc.sync.dma_start(wt[:, 1, :], w0te[P : 2 * P, :])
            xk = constp.tile([P, 2, OWN], BF)
            for s in range(0, W_PER_CORE, NXS):
                n = min(NXS, W_PER_CORE - s) * P
                for kk in range(2):
                    nc.sync.dma_start(
                        xk[:, kk, s * P : s * P + n],
                        xT_own[kk * P : (kk + 1) * P, s * P : s * P + n],
                    )
            estage = constp.tile([P, W_PER_CORE, 8], BF)

            for b in range(0, W_PER_CORE, NB):
                n = min(NB, W_PER_CORE - b)
                hstage = pool.tile([P, NB, IN_F], BF, tag="hstage")
                for i in range(n):
                    m = b + i
                    pu = psum.tile([P, DE], F32, tag="pu")
                    for kk in range(2):
                        nc.tensor.matmul(
                            pu[:],
                            lhsT=xk[:, kk, m * P : (m + 1) * P],
                            rhs=wt[:, kk, :],
                            start=(kk == 0),
                            stop=(kk == 1),
                        )
                    if i % 2 == 0:
                        nc.scalar.copy(hstage[:, i, :], pu[:, 0:IN_F])
                    else:
                        nc.vector.tensor_copy(hstage[:, i, :], pu[:, 0:IN_F])
                    nc.vector.tensor_copy(estage[:, m, :], pu[:, IN_F:DE])
                nc.sync.dma_start(htab_h[:, b : b + n, :], hstage[:, 0:n, :])
            nc.sync.dma_start(htab_e[:], estage[:])
    return nc


def build_k2(b0_zero=False):
    """L0 edge phase + relu + L1 node matmul.

    Inputs (per core), all partition-major:
      hm3  [P, NPAIR, PW] bf16  per pair: he (w2, c, a, h) | meta (w2, c, 8)
                                meta = el[0:4] er[4:8]
      S_in [P, W2, C*128] f8    one-hot dst-selection per window
      b0T  [P, 2] f32           bias by feature-partition ((d,h) order)
      w1te [HF, G] bf16         W1^T | vl1^T | vr1^T  (rows (d,h) order)
      identb [P, P] f32
    Output:
      g_out [P, W2, G] bf16     g | el1 | er1 per node (p, w)

    The L1 stage for pair j-1 is software-pipelined under pair j's edge
    phase so the PE transposes never stall the DVE.
    """
    nc = bass.Bass()
    HF = HEADS * HID                           # 256
    hm3 = nc.dram_tensor("hm3", [P, NPAIR, PW], BF, kind="ExternalInput")
    S_in = nc.dram_tensor("S_in", [P, W2, C * P], F8, kind="ExternalInput")
    b0T = nc.dram_tensor("b0T", [P, 2], F32, kind="ExternalInput")
    w1te = nc.dram_tensor("w1te", [HF, G], BF, kind="ExternalInput")
    identb = nc.dram_tensor("identb", [P, P], F32, kind="ExternalInput")
    g_out = nc.dram_tensor("g_out", [P, W2, G], BF, kind="ExternalOutput")

    DACT = 32   # a-blocks [DACT:64] copied on ACT, [1:DACT] on DVE

    with tile.TileContext(nc) as tc:
        with (
            tc.tile_pool(name="const", bufs=1) as constp,
            tc.tile_pool(name="hmp", bufs=3) as hmp,
            tc.tile_pool(name="sp", bufs=2) as sp,
            tc.tile_pool(name="workp", bufs=2) as workp,
            tc.tile_pool(name="small", bufs=4) as small,
            tc.tile_pool(name="gstp", bufs=2) as gstp,
            tc.tile_pool(name="psum", bufs=2, space="PSUM") as psum,
            tc.tile_pool(name="psum2", bufs=2, space="PSUM") as psum2,
        ):
            b0_sb = constp.tile([P, 2], F32)
            nc.sync.dma_start(b0_sb[:], b0T[:])
            ident_sb = constp.tile([P, P], F32)
            nc.sync.dma_start(ident_sb[:], identb[:])
            w1_sb = constp.tile([P, 2, G], BF)
            nc.sync.dma_start(w1_sb[:, 0, :], w1te[0:P, :])
            nc.sync.dma_start(w1_sb[:, 1, :], w1te[P : 2 * P, :])
            h1T = constp.tile([P, 2, W2 * P], BF)

            S4 = {}
            gst = None
            hm_t = {}
            eex_t = {}
            pu_t = {}
            ptb_t = {}
            pg_t = {}
            for t in range(NPAIR + 4):
                pa = t            # edge elementwise
                pb = t - 1        # multiply + aggregation matmuls
                pc = t - 2        # normalize + transpose
                pd = t - 3        # relu + L1 matmul
                pe = t - 4        # g staging + DMA out

                if pa < NPAIR:
                    if pa % 2 == 0:
                        s4 = sp.tile([P, 4, C * P], F8, tag="S4")
                        nw = min(4, W2 - 2 * pa)
                        nc.sync.dma_start(
                            s4[:, 0:nw, :], S_in[:, 2 * pa : 2 * pa + nw, :]
                        )
                        S4[pa] = s4
                        S4[pa + 1] = s4
                    hm = hmp.tile([P, PW], BF, tag="hm")
                    nc.sync.dma_start(hm[:], hm3[:, pa, :])
                    hm_t[pa] = hm

                    # e = el + er (gpsimd); exp(leaky(e)) = max(exp, exp^.2)
                    mv = hm[:, HE2:PW].rearrange(
                        "p (w c v) -> p w c v", w=2, v=8
                    )
                    e2 = small.tile([P, 2 * C * HEADS], F32, tag="e2")
                    e2v = e2[:].rearrange("p (w c h) -> p w c h", w=2, h=HEADS)
                    nc.gpsimd.tensor_tensor(
                        out=e2v, in0=mv[:, :, :, 0:4], in1=mv[:, :, :, 4:8],
                        op=mybir.AluOpType.add,
                    )
                    exA = small.tile([P, 2 * C * HEADS], BF, tag="exA")
                    nc.scalar.activation(
                        exA[:], e2[:], mybir.ActivationFunctionType.Exp
                    )
                    exB = small.tile([P, 2 * C * HEADS], BF, tag="exB")
                    nc.scalar.activation(
                        exB[:], e2[:], mybir.ActivationFunctionType.Exp,
                        scale=NEG_SLOPE,
                    )
                    # eex (w2, c, a(65), h); a=0 = ee (denominator; he ones)
                    eex = workp.tile([P, NB2, 65, HEADS], BF, tag="eex")
                    eex_t[pa] = eex
                    nc.vector.tensor_tensor(
                        out=eex[:, :, 0, :], in0=exA[:], in1=exB[:],
                        op=mybir.AluOpType.max,
                    )
                    a = 1
                    while a < 16:
                        nc.vector.tensor_copy(
                            eex[:, :, a : 2 * a, :], eex[:, :, 0:a, :]
                        )
                        a *= 2
                    nc.vector.tensor_copy(
                        eex[:, :, 16:24, :], eex[:, :, 0:8, :]
                    )
                    nc.scalar.copy(eex[:, :, 24:48, :], eex[:, :, 0:24, :])
                    nc.scalar.copy(eex[:, :, 48:64, :], eex[:, :, 0:16, :])
                    nc.scalar.copy(eex[:, :, 64:65, :], eex[:, :, 0:1, :])

                # ---- multiply + aggregation for pair pb (eex ready last iter)
                if 0 <= pb < NPAIR:
                    msg = workp.tile([P, NB2, 65, HEADS], BF, tag="msg")
                    nc.vector.tensor_tensor(
                        out=msg[:],
                        in0=hm_t[pb][:, 0:HE2],
                        in1=eex_t.pop(pb)[:],
                        op=mybir.AluOpType.mult,
                    )
                    msgv = msg[:].rearrange("p (w c) a h -> p w c (a h)", w=2)
                    pu2 = psum.tile([P, 2, 512], F32, tag="pu")
                    pu_t[pb] = pu2
                    for jj in range(2):
                        w = 2 * pb + jj
                        jS = w % 4
                        for c in range(C):
                            nc.tensor.matmul(
                                pu2[:, jj, 0 : 65 * HEADS],
                                lhsT=S4[pb][:, jS, c * P : (c + 1) * P],
                                rhs=msgv[:, jj, c, :],
                                start=(c == 0),
                                stop=(c == C - 1),
                            )

                # ---- normalize + transpose for pair pc
                if 0 <= pc < NPAIR:
                    pu_c = pu_t.pop(pc)
                    s_eps = small.tile([P, 2 * HEADS], F32, tag="s_eps")
                    nc.vector.tensor_scalar_add(
                        s_eps[:], pu_c[:, :, 0:HEADS], 1e-38
                    )
                    rs = small.tile([P, 2 * HEADS], F32, tag="rs")
                    nc.vector.reciprocal(rs[:], s_eps[:])
                    h1s2 = small.tile([P, 2, HF], F32, tag="h1s2")
                    nc.vector.tensor_tensor(
                        out=h1s2[:].rearrange("p w (d h) -> p w h d", h=HEADS),
                        in0=pu_c[:, :, 0 : 65 * HEADS].rearrange(
                            "p w (a h) -> p w h a", h=HEADS
                        )[:, :, :, 1:65],
                        in1=rs[:]
                        .rearrange("p (w h) -> p w h", w=2)
                        .to_broadcast([P, 2, HEADS, HID]),
                        op=mybir.AluOpType.mult,
                    )
                    ptb2 = psum2.tile([P, 2, 2, P], F32, tag="ptb")
                    ptb_t[pc] = ptb2
                    for jj in range(2):
                        for kk in range(2):
                            nc.tensor.transpose(
                                out=ptb2[:, jj, kk, :],
                                in_=h1s2[:, jj, kk * P : (kk + 1) * P],
                                identity=ident_sb[:],
                            )

                # ---- relu + L1 matmul for pair pd
                if 0 <= pd < NPAIR:
                    ptb2 = ptb_t.pop(pd)
                    if b0_zero:
                        nc.vector.tensor_scalar_max(
                            h1T[:].rearrange("p k (w n) -> p w k n", n=P)[
                                :, 2 * pd : 2 * pd + 2, :, :
                            ],
                            ptb2[:],
                            0.0,
                        )
                    else:
                        for kk in range(2):
                            nc.vector.tensor_scalar(
                                out=h1T[:, kk, 2 * pd * P : (2 * pd + 2) * P],
                                in0=ptb2[:, :, kk, :],
                                scalar1=b0_sb[:, kk : kk + 1],
                                scalar2=0.0,
                                op0=mybir.AluOpType.add,
                                op1=mybir.AluOpType.max,
                            )
                    pg2 = psum2.tile([P, 2, 256], F32, tag="pg")
                    pg_t[pd] = pg2
                    for jj in range(2):
                        w = 2 * pd + jj
                        for kk in range(2):
                            nc.tensor.matmul(
                                pg2[:, jj, 0:G],
                                lhsT=h1T[:, kk, w * P : (w + 1) * P],
                                rhs=w1_sb[:, kk, :],
                                start=(kk == 0),
                                stop=(kk == 1),
                            )

                # ---- g staging + output DMA for pair pe
                if 0 <= pe < NPAIR:
                    pg2 = pg_t.pop(pe)
                    if pe % 2 == 0:
                        gst = gstp.tile([P, 4, G], BF, tag="gst")
                    slot = 2 * (pe % 2)
                    nc.scalar.copy(gst[:, slot : slot + 2, :], pg2[:, :, 0:G])
                    if pe % 2 == 1 or pe == NPAIR - 1:
                        n = slot + 2
                        w1b = 2 * pe + 2 - n
                        nc.sync.dma_start(
                            g_out[:, w1b : w1b + n, :], gst[:, 0:n, :]
                        )
    return nc


def build_k3():
    """L1 edge phase: y = (sum_e ee1*g[src]) / (sum_e ee1) + b1 per dst node.

    Inputs:
      gm3  [P, NPAIR, PW3] bf16  per pair: g (w2, c, a) | meta (w2, el/er, c)
      S_in [P, W2, C*128] f8
      b1r  [P, OUT_F] f32
    Output:
      y_out [P, W2, OUT_F] f32
    """
    nc = bass.Bass()
    gm3 = nc.dram_tensor("gm3", [P, NPAIR, PW3], BF, kind="ExternalInput")
    S_in = nc.dram_tensor("S_in", [P, W2, C * P], F8, kind="ExternalInput")
    b1r = nc.dram_tensor("b1r", [P, 2 * OUT_F], F32, kind="ExternalInput")
    y_out = nc.dram_tensor("y_out", [P, W2, OUT_F], F32, kind="ExternalOutput")

    DACT = 32

    with tile.TileContext(nc) as tc:
        with (
            tc.tile_pool(name="const", bufs=1) as constp,
            tc.tile_pool(name="gmp", bufs=3) as gmp,
            tc.tile_pool(name="sp", bufs=2) as sp,
            tc.tile_pool(name="workp", bufs=2) as workp,
            tc.tile_pool(name="small", bufs=4) as small,
            tc.tile_pool(name="ystp", bufs=2) as ystp,
            tc.tile_pool(name="psum", bufs=2, space="PSUM") as psum,
        ):
            b1_sb = constp.tile([P, 2 * OUT_F], F32)
            nc.sync.dma_start(b1_sb[:], b1r[:])

            S4 = {}
            yst = None
            gm_t = {}
            eex_t = {}
            pu_t = {}
            for t in range(NPAIR + 2):
                pa = t
                pb = t - 1
                pc = t - 2

                if pa < NPAIR:
                    if pa % 2 == 0:
                        s4 = sp.tile([P, 4, C * P], F8, tag="S4")
                        nw = min(4, W2 - 2 * pa)
                        nc.sync.dma_start(
                            s4[:, 0:nw, :], S_in[:, 2 * pa : 2 * pa + nw, :]
                        )
                        S4[pa] = s4
                        S4[pa + 1] = s4
                    gm = gmp.tile([P, PW3], BF, tag="gm")
                    nc.sync.dma_start(gm[:], gm3[:, pa, :])
                    gm_t[pa] = gm

                    mv = gm[:, GE2:PW3].rearrange(
                        "p (w v c) -> p w v c", w=2, v=2
                    )
                    e1 = small.tile([P, 2 * C], F32, tag="e1")
                    e1v = e1[:].rearrange("p (w c) -> p w c", w=2)
                    nc.gpsimd.tensor_tensor(
                        out=e1v, in0=mv[:, :, 0, :], in1=mv[:, :, 1, :],
                        op=mybir.AluOpType.add,
                    )
                    exA = small.tile([P, 2 * C], BF, tag="exA")
                    nc.scalar.activation(
                        exA[:], e1[:], mybir.ActivationFunctionType.Exp
                    )
                    exB = small.tile([P, 2 * C], BF, tag="exB")
                    nc.scalar.activation(
                        exB[:], e1[:], mybir.ActivationFunctionType.Exp,
                        scale=NEG_SLOPE,
                    )
                    eex = workp.tile([P, NB2, 65], BF, tag="eex")
                    eex_t[pa] = eex
                    nc.vector.tensor_tensor(
                        out=eex[:, :, 0], in0=exA[:], in1=exB[:],
                        op=mybir.AluOpType.max,
                    )
                    a = 1
                    while a < 16:
                        nc.vector.tensor_copy(
                            eex[:, :, a : 2 * a], eex[:, :, 0:a]
                        )
                        a *= 2
                    nc.vector.tensor_copy(eex[:, :, 16:24], eex[:, :, 0:8])
                    nc.scalar.copy(eex[:, :, 24:48], eex[:, :, 0:24])
                    nc.scalar.copy(eex[:, :, 48:64], eex[:, :, 0:16])
                    nc.scalar.copy(eex[:, :, 64:65], eex[:, :, 0:1])

                # ---- multiply + aggregation for pair pb
                if 0 <= pb < NPAIR:
                    msg = workp.tile([P, NB2, 65], BF, tag="msg")
                    nc.vector.tensor_tensor(
                        out=msg[:], in0=gm_t[pb][:, 0:GE2],
                        in1=eex_t.pop(pb)[:],
                        op=mybir.AluOpType.mult,
                    )
                    msgv = msg[:].rearrange("p (w c) a -> p w c a", w=2)
                    pu2 = psum.tile([P, 2, 256], F32, tag="pu")
                    pu_t[pb] = pu2
                    for jj in range(2):
                        w = 2 * pb + jj
                        jS = w % 4
                        for c in range(C):
                            nc.tensor.matmul(
                                pu2[:, jj, 0:65],
                                lhsT=S4[pb][:, jS, c * P : (c + 1) * P],
                                rhs=msgv[:, jj, c, :],
                                start=(c == 0),
                                stop=(c == C - 1),
                            )

                # ---- output stage for pair pc
                if 0 <= pc < NPAIR:
                    pu_c = pu_t.pop(pc)
                    s_eps = small.tile([P, 2], F32, tag="s_eps")
                    nc.vector.tensor_scalar_add(s_eps[:], pu_c[:, :, 0], 1e-38)
                    rs = small.tile([P, 2], F32, tag="rs")
                    nc.vector.reciprocal(rs[:], s_eps[:])
                    if pc % 2 == 0:
                        yst = ystp.tile([P, 4, OUT_F], F32, tag="yst")
                    slot = 2 * (pc % 2)
                    ysb = yst[:, slot : slot + 2, :]
                    nc.vector.tensor_tensor(
                        out=ysb,
                        in0=pu_c[:, :, 1 : OUT_F + 1],
                        in1=rs[:].to_broadcast([P, 2, OUT_F]),
                        op=mybir.AluOpType.mult,
                    )
                    nc.vector.tensor_tensor(
                        out=ysb,
                        in0=ysb,
                        in1=b1_sb[:].rearrange("p (w f) -> p w f", w=2),
                        op=mybir.AluOpType.add,
                    )
                    if pc % 2 == 1 or pc == NPAIR - 1:
                        n = slot + 2
                        w1b = 2 * pc + 2 - n
                        nc.sync.dma_start(
                            y_out[:, w1b : w1b + n, :], yst[:, 0:n, :]
                        )
    return nc


# ------------------------------------------------------------- host helpers
def _run(nc, in_maps, label):
    profile = os.environ.get("GAT_PROFILE", "0") == "1"
    res = run_bass_kernel_spmd(
        nc, in_maps, core_ids=list(range(NC_CORES)), trace=profile
    )
    if profile:
        EXEC_TIMES_NS[label] = res.exec_time_ns
    return res.results


def _pack_nodes(dst):
    """Degree-balanced node->window permutation.  Returns perm [PADN] where
    perm[b*128 + p] = original node id placed at (bin b, position p);
    every bin's in-degree sum is <= C*128."""
    deg = np.bincount(dst, minlength=PADN).astype(np.int64)
    NBINS = NC_CORES * W_PER_CORE
    CAP = C * P
    order = np.argsort(-deg, kind="stable")
    arr = order.reshape(P, NBINS).copy()
    arr[1::2] = arr[1::2, ::-1]
    loads = deg[arr].sum(axis=0)
    nodes_by_bin = np.ascontiguousarray(arr.T)          # [NBINS, P]
    it = 0
    while loads.max() > CAP and it < 20000:
        b_hi = int(np.argmax(loads))
        b_lo = int(np.argmin(loads))
        excess = loads[b_hi] - CAP
        dh = deg[nodes_by_bin[b_hi]]
        dl = deg[nodes_by_bin[b_lo]]
        diff = dh[:, None] - dl[None, :]
        diff = np.where(diff > 0, diff, 10**9)
        i, j = np.unravel_index(np.argmin(np.abs(diff - excess)), diff.shape)
        d = dh[i] - dl[j]
        if d <= 0:
            break
        nodes_by_bin[b_hi, i], nodes_by_bin[b_lo, j] = (
            nodes_by_bin[b_lo, j],
            nodes_by_bin[b_hi, i],
        )
        loads[b_hi] -= d
        loads[b_lo] += d
        it += 1
    assert loads.max() <= CAP, f"window packing failed: max={loads.max()}"
    return nodes_by_bin.reshape(-1)


def _edge_slots(src, dst, packed_of):
    """Per-core edge->slot assignment in packed id space.  Returns
    (sidx, ddst, dloc): int64 [NC, W2, C*128] (pad = PADN), dloc f32."""
    psrc = packed_of[src]
    pdst = packed_of[dst]
    core = pdst // OWN
    win = (pdst % OWN) // P
    loc = pdst % P

    order = np.lexsort((win, core))
    s_src, s_core, s_win, s_loc = psrc[order], core[order], win[order], loc[order]
    group = s_core * W_PER_CORE + s_win
    cnt = np.bincount(group, minlength=NC_CORES * W_PER_CORE)
    gstart = np.zeros(NC_CORES * W_PER_CORE, dtype=np.int64)
    gstart[1:] = np.cumsum(cnt)[:-1]
    within = np.arange(len(order)) - gstart[group]
    assert within.max() < C * P

    sidx = np.full((NC_CORES, W2, C * P), PADN, dtype=np.int64)
    ddst = np.full((NC_CORES, W2, C * P), PADN, dtype=np.int64)
    dloc = np.full((NC_CORES, W2, C * P), -1.0, dtype=np.float32)
    sidx[s_core, s_win, within] = s_src
    ddst[s_core, s_win, within] = s_core * OWN + s_win * P + s_loc
    dloc[s_core, s_win, within] = s_loc.astype(np.float32)
    return sidx, ddst, dloc


def kernel(x, src, dst, W0, al0, ar0, b0, W1, al1, ar1, b1):
    _patch_tile()
    _install_ntff_hook()
    import ml_dtypes

    BFh = ml_dtypes.bfloat16
    F8h = ml_dtypes.float8_e4m3

    x = np.asarray(x, dtype=np.float32)
    src = np.asarray(src, dtype=np.int64)
    dst = np.asarray(dst, dtype=np.int64)
    W0 = np.asarray(W0, dtype=np.float32)
    al0 = np.asarray(al0, dtype=np.float32)
    ar0 = np.asarray(ar0, dtype=np.float32)
    b0 = np.asarray(b0, dtype=np.float32)
    W1 = np.asarray(W1, dtype=np.float32)
    al1 = np.asarray(al1, dtype=np.float32)
    ar1 = np.asarray(ar1, dtype=np.float32)
    b1 = np.asarray(b1, dtype=np.float32)

    HF = HEADS * HID

    # ---- weight prep
    vl0 = np.einsum("hd,hdk->hk", al0, W0.reshape(HEADS, HID, IN_F))
    vr0 = np.einsum("hd,hdk->hk", ar0, W0.reshape(HEADS, HID, IN_F))
    w0te = np.concatenate([W0.T, vl0.T, vr0.T], axis=1).astype(BFh)  # [256, 264]
    vl1 = al1 @ W1
    vr1 = ar1 @ W1
    w1te_orig = np.concatenate([W1.T, vl1.T, vr1.T], axis=1)         # [256, 66]
    # reorder features to the kernel's (d, h) layout
    jidx = np.arange(HF)
    feat_orig = (jidx % HEADS) * HID + jidx // HEADS
    w1te = w1te_orig[feat_orig].astype(BFh)
    b0T = np.ascontiguousarray(
        b0[feat_orig].reshape(2, P).T
    ).astype(np.float32)                                             # [128, 2]
    b1r = np.tile(b1[None, :], (P, 2)).astype(np.float32)

    # ---- node packing permutation
    perm = _pack_nodes(dst)                     # [PADN] packed -> orig
    packed_of = np.empty(PADN, dtype=np.int64)
    packed_of[perm] = np.arange(PADN)

    x_bf = x.astype(BFh)
    xg = np.zeros((PADN, IN_F), dtype=BFh)
    valid = perm < N_NODES
    xg[valid] = x_bf[perm[valid]]
    xT_pad = np.ascontiguousarray(xg.T)         # [256, PADN] packed order

    identb = np.eye(P, dtype=np.float32)

    # ---- K1: node tables
    nc1 = build_k1()
    in1 = [
        {
            "xT_own": np.ascontiguousarray(xT_pad[:, k * OWN : (k + 1) * OWN]),
            "w0te": w0te,
        }
        for k in range(NC_CORES)
    ]
    r1 = _run(nc1, in1, "k1")
    htab_h = np.concatenate(
        [r1[k]["htab_h"].transpose(1, 0, 2).reshape(OWN, IN_F) for k in range(NC_CORES)],
        axis=0,
    )
    htab_e = np.concatenate(
        [r1[k]["htab_e"].transpose(1, 0, 2).reshape(OWN, 8) for k in range(NC_CORES)],
        axis=0,
    )

    # ---- edge layout
    sidx, ddst, dloc = _edge_slots(src, dst, packed_of)

    htab_h_x = np.concatenate([htab_h, np.zeros((1, IN_F), dtype=BFh)], axis=0)
    htab_e_x = np.concatenate([htab_e, np.zeros((1, 8), dtype=BFh)], axis=0)

    # one-hot tiles: S[p, w, c*128+n] = (dloc == n), fp8
    def s_tiles(dl):
        d3 = dl.reshape(W2, C, P)
        oh = d3[:, :, :, None] == np.arange(P, dtype=np.float32)[None, None, None, :]
        return np.ascontiguousarray(
            oh.transpose(2, 0, 1, 3).reshape(P, W2, C * P).astype(F8h)
        )

    # ---- K2 inputs
    nc2 = build_k2(b0_zero=bool(np.all(b0 == 0.0)))
    in2 = []
    for k in range(NC_CORES):
        hg = htab_h_x[sidx[k]]                  # [W2, C*P, 256] bf16
        # -> he3 [P, pair, w2, c, a(65), h]; a=0 is all-ones (denominator),
        #    a=1+d holds hg[w, c*128+p, h*64+d]
        he_feat = (
            hg.reshape(NPAIR, 2, C, P, HEADS, HID)
            .transpose(3, 0, 1, 2, 5, 4)        # (p, pair, w2, c, d, h)
        )
        he3 = np.concatenate(
            [np.ones((P, NPAIR, 2, C, 1, HEADS), dtype=BFh), he_feat], axis=4
        ).reshape(P, NPAIR, HE2)
        meta = np.empty((W2, C * P, 8), dtype=BFh)
        meta[:, :, 0:4] = htab_e_x[sidx[k], 0:4]
        meta[:, :, 4:8] = htab_e_x[ddst[k], 4:8]
        meta3 = (
            meta.reshape(NPAIR, 2, C, P, 8)
            .transpose(3, 0, 1, 2, 4)           # (p, pair, w2, c, v)
            .reshape(P, NPAIR, MW2)
        )
        hm3 = np.concatenate([he3, meta3], axis=2)
        in2.append(
            {
                "hm3": np.ascontiguousarray(hm3),
                "S_in": s_tiles(dloc[k]),
                "b0T": b0T,
                "w1te": w1te,
                "identb": identb,
            }
        )
    r2 = _run(nc2, in2, "k2")
    gtab = np.concatenate(
        [
            r2[k]["g_out"][:, :W_PER_CORE, :].transpose(1, 0, 2).reshape(OWN, G)
            for k in range(NC_CORES)
        ],
        axis=0,
    )
    gtab_x = np.concatenate([gtab, np.zeros((1, G), dtype=gtab.dtype)], axis=0)

    # ---- K3 inputs
    nc3 = build_k3()
    in3 = []
    for k in range(NC_CORES):
        gg = gtab_x[sidx[k], :OUT_F]            # [W2, C*P, 64]
        g_feat = (
            gg.reshape(NPAIR, 2, C, P, OUT_F)
            .transpose(3, 0, 1, 2, 4)           # (p, pair, w2, c, a)
        )
        g3 = np.concatenate(
            [np.ones((P, NPAIR, 2, C, 1), dtype=BFh), g_feat], axis=4
        ).reshape(P, NPAIR, GE2)
        m1 = np.empty((W2, C * P, 2), dtype=BFh)
        m1[:, :, 0] = gtab_x[sidx[k], OUT_F]
        m1[:, :, 1] = gtab_x[ddst[k], OUT_F + 1]
        meta13 = (
            m1.reshape(NPAIR, 2, C, P, 2)
            .transpose(3, 0, 1, 4, 2)           # (p, pair, w2, v, c)
            .reshape(P, NPAIR, MW3)
        )
        gm3 = np.concatenate([g3, meta13], axis=2)
        in3.append(
            {
                "gm3": np.ascontiguousarray(gm3),
                "S_in": in2[k]["S_in"],
                "b1r": b1r,
            }
        )
    r3 = _run(nc3, in3, "k3")
    y_packed = np.concatenate(
        [
            r3[k]["y_out"][:, :W_PER_CORE, :].transpose(1, 0, 2).reshape(OWN, OUT_F)
            for k in range(NC_CORES)
        ],
        axis=0,
    )
    y_full = np.empty((PADN, OUT_F), dtype=np.float32)
    y_full[perm] = y_packed.astype(np.float32)
    return np.ascontiguousarray(y_full[:N_NODES])


# revision 38
# speedup vs baseline: 1.0817x; 1.0817x over previous
"""Two-layer GAT (4-head then 1-head) on 8 NeuronCores.

Sharding: nodes are partitioned across the 8 cores by dst-ownership after a
degree-balancing permutation (snake-deal of degree-sorted nodes into 392
windows of 128 nodes, max in-degree per window <= 2048 = C*128 with C=16).
Each core processes all edges whose dst it owns.  Per-dst-window the segment
softmax + weighted aggregation run as one-hot-selection matmuls on the tensor
engine.

Key kernel techniques vs the naive formulation:
  - per-edge attention exp() is computed on the small [slot, w2*c*h] tile,
    then broadcast across the 64 features by log2-doubling copies whose
    inner runs are contiguous (layout (w2, c, a, h) with (w2, c) outermost),
    split between the DVE and ACT engines;
  - matmul rhs slices [128, 260] are fully contiguous per (window, chunk);
  - windows are processed in pairs so elementwise instruction overheads are
    amortized;
  - all big DMAs are partition-major single-run transfers batched over
    multiple windows;
  - h1 is re-transposed through the PE and layer-1 bias+relu run as a single
    fused two-op tensor_scalar on the DVE; the ACT engine only ever runs Exp
    and Copy so activation-table reloads are minimized.

Three SPMD launches:
  K1: h|el|er = x @ [W0^T | vl0^T | vr0^T]   (node-sharded, bf16)
  K2: L0 edge phase (attention + aggregation) + relu + L1 node matmul
  K3: L1 edge phase -> output

Between launches the host performs pure index gathers / dtype casts of
device-computed tables; all floating-point math runs on device.
"""
import os
import sys
import types

sys.path.insert(0, "/opt/trn_rl_repo")

import numpy as np

import concourse.bass as bass
import concourse.tile as tile
from concourse import mybir
from concourse.bass_utils import run_bass_kernel_spmd
from concourse.vector_clock import ScopedClock

# ---------------------------------------------------------------- constants
N_NODES = 50000
IN_F = 256
HID = 64
HEADS = 4
OUT_F = 64
NEG_SLOPE = 0.2

NC_CORES = 8
P = 128
W_PER_CORE = 49
OWN = W_PER_CORE * P            # 6272 nodes per core
PADN = NC_CORES * OWN           # 50176
C = 16                          # edge chunks per window (cap C*128 = 2048)
W2 = 50                         # windows padded to even count for pairing
NPAIR = W2 // 2                 # 25

F32 = mybir.dt.float32
BF = mybir.dt.bfloat16
F8 = mybir.dt.float8e4

NB2 = 2 * C                     # 32 (w2, c) blocks per pair, layer 0/1
HE2 = NB2 * 65 * HEADS          # 8320  he cols per pair (a=0 ones | a=1..64)
MW2 = 2 * C * 8                 # 256   meta cols per pair
PW = HE2 + MW2                  # 8576

GE2 = NB2 * 65                  # 2080
MW3 = 2 * 2 * C                 # 64
PW3 = GE2 + MW3                 # 2144

G = OUT_F + 2                   # 66

EXEC_TIMES_NS = {}              # filled when GAT_PROFILE=1


# ------------------------------------------------------------- tile patches
def _patch_tile():
    """This container's walrus rejects instructions with >1 sem wait
    ("Too many sync wait commands").  After Tile lowering, move excess waits
    onto same-engine no-ops inserted before the offending instruction."""
    if getattr(_patch_tile, "done", False):
        return
    _patch_tile.done = True

    MAX_WAITS = 1

    def _split_all_waits(nc):
        for bb in nc.main_func.blocks:
            insts = bb.instructions
            i = 0
            while i < len(insts):
                inst = insts[i]
                si = getattr(inst, "sync_info", None)
                if si is None or len(si.on_wait) <= MAX_WAITS:
                    i += 1
                    continue
                waits = list(si.on_wait)
                si.on_wait[:] = waits[:MAX_WAITS]
                extra = waits[MAX_WAITS:]
                nops = []
                for j in range(0, len(extra), MAX_WAITS):
                    nop = mybir.InstNoOp(
                        name=f"I-waitsplit-{nc.next_id()}",
                        ins=[],
                        outs=[],
                        engine=inst.engine,
                    )
                    nop.sync_info = mybir.SyncInfo(
                        on_wait=extra[j : j + MAX_WAITS], on_update=[]
                    )
                    nc.register_instruction(nop, overwrite=True)
                    nops.append(nop)
                insts[i:i] = nops
                i += len(nops) + 1

    def _drain_and_barrier(self, tick_clock, wait_clock):
        drain_inst = self.nc.sync.drain()
        wait_clock.add_sem_waits(
            drain_inst.ins, ScopedClock({None: tick_clock.global_clock})
        )
        self.nc.all_engine_barrier()
        assert self.sems is not None
        popped = self.nc._tile_sem_poison_stack.pop()
        assert popped is self._sem_poison
        self.nc.clear_and_free_semaphores(list(self.sems.allocated().values()))
        self.nc.all_engine_barrier()
        _split_all_waits(self.nc)

    tile.TileContext._drain_and_barrier = _drain_and_barrier


def _install_ntff_hook():
    """Enable run_bass_kernel_spmd(trace=True) under axon: register the NTFF
    profile hook that the boot script skips when antenv.axon_hooks is absent."""
    if getattr(_install_ntff_hook, "done", False):
        return
    _install_ntff_hook.done = True
    try:
        mod = types.ModuleType("antenv.axon_hooks")
        _state = {}

        def set_axon_ntff_profile_hook(h):
            _state["h"] = h

        def get_axon_ntff_profile_hook():
            return _state.get("h")

        mod.set_axon_ntff_profile_hook = set_axon_ntff_profile_hook
        mod.get_axon_ntff_profile_hook = get_axon_ntff_profile_hook
        sys.modules["antenv.axon_hooks"] = mod
        import antenv

        antenv.axon_hooks = mod
        from trn_agent_boot.trn_boot import _ntff_profile_via_ctypes

        hook = _ntff_profile_via_ctypes("/opt/axon/libaxon_pjrt.so")
        if hook is not None:
            set_axon_ntff_profile_hook(hook)
    except Exception:
        pass


# ------------------------------------------------------------- kernel builders
def build_k1():
    """h|el|er tables for this core's 6272 nodes (bf16 matmul + bf16 out)."""
    nc = bass.Bass()
    DE = IN_F + 2 * HEADS                     # 264
    xT_own = nc.dram_tensor("xT_own", [IN_F, OWN], BF, kind="ExternalInput")
    w0te = nc.dram_tensor("w0te", [IN_F, DE], BF, kind="ExternalInput")
    htab_h = nc.dram_tensor("htab_h", [P, W_PER_CORE, IN_F], BF, kind="ExternalOutput")
    htab_e = nc.dram_tensor("htab_e", [P, W_PER_CORE, 8], BF, kind="ExternalOutput")

    NB = 7                                    # output batch (49 = 7*7)
    NXS = 7                                   # xk loaded in 7-window slabs
    with tile.TileContext(nc) as tc:
        with (
            tc.tile_pool(name="const", bufs=1) as constp,
            tc.tile_pool(name="sbuf", bufs=2) as pool,
            tc.tile_pool(name="psum", bufs=4, space="PSUM") as psum,
        ):
            wt = constp.tile([P, 2, DE], BF)
            nc.sync.dma_start(wt[:, 0, :], w0te[0:P, :])
            nc.sync.dma_start(wt[:, 1, :], w0te[P : 2 * P, :])
            xk = constp.tile([P, 2, OWN], BF)
            for s in range(0, W_PER_CORE, NXS):
                n = min(NXS, W_PER_CORE - s) * P
                for kk in range(2):
                    nc.sync.dma_start(
                        xk[:, kk, s * P : s * P + n],
                        xT_own[kk * P : (kk + 1) * P, s * P : s * P + n],
                    )
            estage = constp.tile([P, W_PER_CORE, 8], BF)

            for b in range(0, W_PER_CORE, NB):
                n = min(NB, W_PER_CORE - b)
                hstage = pool.tile([P, NB, IN_F], BF, tag="hstage")
                for i in range(n):
                    m = b + i
                    pu = psum.tile([P, DE], F32, tag="pu")
                    for kk in range(2):
                        nc.tensor.matmul(
                            pu[:],
                            lhsT=xk[:, kk, m * P : (m + 1) * P],
                            rhs=wt[:, kk, :],
                            start=(kk == 0),
                            stop=(kk == 1),
                        )
                    if i % 2 == 0:
                        nc.scalar.copy(hstage[:, i, :], pu[:, 0:IN_F])
                    else:
                        nc.vector.tensor_copy(hstage[:, i, :], pu[:, 0:IN_F])
                    nc.vector.tensor_copy(estage[:, m, :], pu[:, IN_F:DE])
                nc.sync.dma_start(htab_h[:, b : b + n, :], hstage[:, 0:n, :])
            nc.sync.dma_start(htab_e[:], estage[:])
    return nc


def build_k2(b0_zero=False):
    """L0 edge phase + relu + L1 node matmul.

    Inputs (per core), all partition-major:
      hm3  [P, NPAIR, PW] bf16  per pair: he (w2, c, a, h) | meta (w2, c, 8)
                                meta = el[0:4] er[4:8]
      S_in [P, W2, C*128] f8    one-hot dst-selection per window
      b0T  [P, 2] f32           bias by feature-partition ((d,h) order)
      w1te [HF, G] bf16         W1^T | vl1^T | vr1^T  (rows (d,h) order)
      identb [P, P] f32
    Output:
      g_out [P, W2, G] bf16     g | el1 | er1 per node (p, w)

    The L1 stage for pair j-1 is software-pipelined under pair j's edge
    phase so the PE transposes never stall the DVE.
    """
    nc = bass.Bass()
    HF = HEADS * HID                           # 256
    hm3 = nc.dram_tensor("hm3", [P, NPAIR, PW], BF, kind="ExternalInput")
    S_in = nc.dram_tensor("S_in", [P, W2, C * P], F8, kind="ExternalInput")
    b0T = nc.dram_tensor("b0T", [P, 2], F32, kind="ExternalInput")
    w1te = nc.dram_tensor("w1te", [HF, G], BF, kind="ExternalInput")
    identb = nc.dram_tensor("identb", [P, P], F32, kind="ExternalInput")
    g_out = nc.dram_tensor("g_out", [P, W2, G], BF, kind="ExternalOutput")

    DACT = 32   # a-blocks [DACT:64] copied on ACT, [1:DACT] on DVE

    with tile.TileContext(nc) as tc:
        with (
            tc.tile_pool(name="const", bufs=1) as constp,
            tc.tile_pool(name="hmp", bufs=4) as hmp,
            tc.tile_pool(name="sp", bufs=2) as sp,
            tc.tile_pool(name="workp", bufs=2) as workp,
            tc.tile_pool(name="small", bufs=4) as small,
            tc.tile_pool(name="gstp", bufs=2) as gstp,
            tc.tile_pool(name="psum", bufs=2, space="PSUM") as psum,
            tc.tile_pool(name="psum2", bufs=2, space="PSUM") as psum2,
        ):
            b0_sb = constp.tile([P, 2], F32)
            nc.sync.dma_start(b0_sb[:], b0T[:])
            ident_sb = constp.tile([P, P], F32)
            nc.sync.dma_start(ident_sb[:], identb[:])
            w1_sb = constp.tile([P, 2, G], BF)
            nc.sync.dma_start(w1_sb[:, 0, :], w1te[0:P, :])
            nc.sync.dma_start(w1_sb[:, 1, :], w1te[P : 2 * P, :])
            h1T = constp.tile([P, 2, W2 * P], BF)

            S4 = {}
            gst = None
            hm_t = {}
            eex_t = {}
            pu_t = {}
            ptb_t = {}
            pg_t = {}
            for t in range(NPAIR + 5):
                pf = t            # input DMA prefetch
                pa = t - 1        # edge elementwise
                pb = t - 2        # multiply + aggregation matmuls
                pc = t - 3        # normalize + transpose
                pd = t - 4        # relu + L1 matmul
                pe = t - 5        # g staging + DMA out

                if pf < NPAIR:
                    if pf % 2 == 0:
                        s4 = sp.tile([P, 4, C * P], F8, tag="S4")
                        nw = min(4, W2 - 2 * pf)
                        nc.sync.dma_start(
                            s4[:, 0:nw, :], S_in[:, 2 * pf : 2 * pf + nw, :]
                        )
                        S4[pf] = s4
                        S4[pf + 1] = s4
                    hm = hmp.tile([P, PW], BF, tag="hm")
                    nc.sync.dma_start(hm[:], hm3[:, pf, :])
                    hm_t[pf] = hm

                if 0 <= pa < NPAIR:
                    hm = hm_t[pa]
                    # e = el + er (gpsimd); exp(leaky(e)) = max(exp, exp^.2)
                    mv = hm[:, HE2:PW].rearrange(
                        "p (w c v) -> p w c v", w=2, v=8
                    )
                    e2 = small.tile([P, 2 * C * HEADS], F32, tag="e2")
                    e2v = e2[:].rearrange("p (w c h) -> p w c h", w=2, h=HEADS)
                    nc.gpsimd.tensor_tensor(
                        out=e2v, in0=mv[:, :, :, 0:4], in1=mv[:, :, :, 4:8],
                        op=mybir.AluOpType.add,
                    )
                    exA = small.tile([P, 2 * C * HEADS], BF, tag="exA")
                    nc.scalar.activation(
                        exA[:], e2[:], mybir.ActivationFunctionType.Exp
                    )
                    exB = small.tile([P, 2 * C * HEADS], BF, tag="exB")
                    nc.scalar.activation(
                        exB[:], e2[:], mybir.ActivationFunctionType.Exp,
                        scale=NEG_SLOPE,
                    )
                    # eex (w2, c, a(65), h); a=0 = ee (denominator; he ones)
                    eex = workp.tile([P, NB2, 65, HEADS], BF, tag="eex")
                    eex_t[pa] = eex
                    nc.vector.tensor_tensor(
                        out=eex[:, :, 0, :], in0=exA[:], in1=exB[:],
                        op=mybir.AluOpType.max,
                    )
                    a = 1
                    while a < 16:
                        nc.vector.tensor_copy(
                            eex[:, :, a : 2 * a, :], eex[:, :, 0:a, :]
                        )
                        a *= 2
                    nc.vector.tensor_copy(
                        eex[:, :, 16:24, :], eex[:, :, 0:8, :]
                    )
                    nc.scalar.copy(eex[:, :, 24:48, :], eex[:, :, 0:24, :])
                    nc.scalar.copy(eex[:, :, 48:64, :], eex[:, :, 0:16, :])
                    nc.scalar.copy(eex[:, :, 64:65, :], eex[:, :, 0:1, :])

                # ---- multiply + aggregation for pair pb (eex ready last iter)
                if 0 <= pb < NPAIR:
                    msg = workp.tile([P, NB2, 65, HEADS], BF, tag="msg")
                    nc.vector.tensor_tensor(
                        out=msg[:],
                        in0=hm_t[pb][:, 0:HE2],
                        in1=eex_t.pop(pb)[:],
                        op=mybir.AluOpType.mult,
                    )
                    msgv = msg[:].rearrange("p (w c) a h -> p w c (a h)", w=2)
                    pu2 = psum.tile([P, 2, 512], F32, tag="pu")
                    pu_t[pb] = pu2
                    for jj in range(2):
                        w = 2 * pb + jj
                        jS = w % 4
                        for c in range(C):
                            nc.tensor.matmul(
                                pu2[:, jj, 0 : 65 * HEADS],
                                lhsT=S4[pb][:, jS, c * P : (c + 1) * P],
                                rhs=msgv[:, jj, c, :],
                                start=(c == 0),
                                stop=(c == C - 1),
                            )

                # ---- normalize + transpose for pair pc
                if 0 <= pc < NPAIR:
                    pu_c = pu_t.pop(pc)
                    s_eps = small.tile([P, 2 * HEADS], F32, tag="s_eps")
                    nc.vector.tensor_scalar_add(
                        s_eps[:], pu_c[:, :, 0:HEADS], 1e-38
                    )
                    rs = small.tile([P, 2 * HEADS], F32, tag="rs")
                    nc.vector.reciprocal(rs[:], s_eps[:])
                    h1s2 = small.tile([P, 2, HF], F32, tag="h1s2")
                    nc.vector.tensor_tensor(
                        out=h1s2[:].rearrange("p w (d h) -> p w h d", h=HEADS),
                        in0=pu_c[:, :, 0 : 65 * HEADS].rearrange(
                            "p w (a h) -> p w h a", h=HEADS
                        )[:, :, :, 1:65],
                        in1=rs[:]
                        .rearrange("p (w h) -> p w h", w=2)
                        .to_broadcast([P, 2, HEADS, HID]),
                        op=mybir.AluOpType.mult,
                    )
                    ptb2 = psum2.tile([P, 2, 2, P], F32, tag="ptb")
                    ptb_t[pc] = ptb2
                    for jj in range(2):
                        for kk in range(2):
                            nc.tensor.transpose(
                                out=ptb2[:, jj, kk, :],
                                in_=h1s2[:, jj, kk * P : (kk + 1) * P],
                                identity=ident_sb[:],
                            )

                # ---- relu + L1 matmul for pair pd
                if 0 <= pd < NPAIR:
                    ptb2 = ptb_t.pop(pd)
                    if b0_zero:
                        nc.vector.tensor_scalar_max(
                            h1T[:].rearrange("p k (w n) -> p w k n", n=P)[
                                :, 2 * pd : 2 * pd + 2, :, :
                            ],
                            ptb2[:],
                            0.0,
                        )
                    else:
                        for kk in range(2):
                            nc.vector.tensor_scalar(
                                out=h1T[:, kk, 2 * pd * P : (2 * pd + 2) * P],
                                in0=ptb2[:, :, kk, :],
                                scalar1=b0_sb[:, kk : kk + 1],
                                scalar2=0.0,
                                op0=mybir.AluOpType.add,
                                op1=mybir.AluOpType.max,
                            )
                    pg2 = psum2.tile([P, 2, 256], F32, tag="pg")
                    pg_t[pd] = pg2
                    for jj in range(2):
                        w = 2 * pd + jj
                        for kk in range(2):
                            nc.tensor.matmul(
                                pg2[:, jj, 0:G],
                                lhsT=h1T[:, kk, w * P : (w + 1) * P],
                                rhs=w1_sb[:, kk, :],
                                start=(kk == 0),
                                stop=(kk == 1),
                            )

                # ---- g staging + output DMA for pair pe
                if 0 <= pe < NPAIR:
                    pg2 = pg_t.pop(pe)
                    if pe % 2 == 0:
                        gst = gstp.tile([P, 4, G], BF, tag="gst")
                    slot = 2 * (pe % 2)
                    nc.scalar.copy(gst[:, slot : slot + 2, :], pg2[:, :, 0:G])
                    if pe % 2 == 1 or pe == NPAIR - 1:
                        n = slot + 2
                        w1b = 2 * pe + 2 - n
                        nc.sync.dma_start(
                            g_out[:, w1b : w1b + n, :], gst[:, 0:n, :]
                        )
    return nc


def build_k3():
    """L1 edge phase: y = (sum_e ee1*g[src]) / (sum_e ee1) + b1 per dst node.

    Inputs:
      gm3  [P, NPAIR, PW3] bf16  per pair: g (w2, c, a) | meta (w2, el/er, c)
      S_in [P, W2, C*128] f8
      b1r  [P, OUT_F] f32
    Output:
      y_out [P, W2, OUT_F] f32
    """
    nc = bass.Bass()
    gm3 = nc.dram_tensor("gm3", [P, NPAIR, PW3], BF, kind="ExternalInput")
    S_in = nc.dram_tensor("S_in", [P, W2, C * P], F8, kind="ExternalInput")
    b1r = nc.dram_tensor("b1r", [P, 2 * OUT_F], F32, kind="ExternalInput")
    y_out = nc.dram_tensor("y_out", [P, W2, OUT_F], F32, kind="ExternalOutput")

    DACT = 32

    with tile.TileContext(nc) as tc:
        with (
            tc.tile_pool(name="const", bufs=1) as constp,
            tc.tile_pool(name="gmp", bufs=4) as gmp,
            tc.tile_pool(name="sp", bufs=2) as sp,
            tc.tile_pool(name="workp", bufs=2) as workp,
            tc.tile_pool(name="small", bufs=4) as small,
            tc.tile_pool(name="ystp", bufs=2) as ystp,
            tc.tile_pool(name="psum", bufs=2, space="PSUM") as psum,
        ):
            b1_sb = constp.tile([P, 2 * OUT_F], F32)
            nc.sync.dma_start(b1_sb[:], b1r[:])

            S4 = {}
            yst = None
            gm_t = {}
            eex_t = {}
            pu_t = {}
            for t in range(NPAIR + 3):
                pf = t
                pa = t - 1
                pb = t - 2
                pc = t - 3

                if pf < NPAIR:
                    if pf % 2 == 0:
                        s4 = sp.tile([P, 4, C * P], F8, tag="S4")
                        nw = min(4, W2 - 2 * pf)
                        nc.sync.dma_start(
                            s4[:, 0:nw, :], S_in[:, 2 * pf : 2 * pf + nw, :]
                        )
                        S4[pf] = s4
                        S4[pf + 1] = s4
                    gm = gmp.tile([P, PW3], BF, tag="gm")
                    nc.sync.dma_start(gm[:], gm3[:, pf, :])
                    gm_t[pf] = gm

                if 0 <= pa < NPAIR:
                    gm = gm_t[pa]
                    mv = gm[:, GE2:PW3].rearrange(
                        "p (w v c) -> p w v c", w=2, v=2
                    )
                    e1 = small.tile([P, 2 * C], F32, tag="e1")
                    e1v = e1[:].rearrange("p (w c) -> p w c", w=2)
                    nc.gpsimd.tensor_tensor(
                        out=e1v, in0=mv[:, :, 0, :], in1=mv[:, :, 1, :],
                        op=mybir.AluOpType.add,
                    )
                    exA = small.tile([P, 2 * C], BF, tag="exA")
                    nc.scalar.activation(
                        exA[:], e1[:], mybir.ActivationFunctionType.Exp
                    )
                    exB = small.tile([P, 2 * C], BF, tag="exB")
                    nc.scalar.activation(
                        exB[:], e1[:], mybir.ActivationFunctionType.Exp,
                        scale=NEG_SLOPE,
                    )
                    eex = workp.tile([P, NB2, 65], BF, tag="eex")
                    eex_t[pa] = eex
                    nc.vector.tensor_tensor(
                        out=eex[:, :, 0], in0=exA[:], in1=exB[:],
                        op=mybir.AluOpType.max,
                    )
                    a = 1
                    while a < 16:
                        nc.vector.tensor_copy(
                            eex[:, :, a : 2 * a], eex[:, :, 0:a]
                        )
                        a *= 2
                    nc.vector.tensor_copy(eex[:, :, 16:24], eex[:, :, 0:8])
                    nc.scalar.copy(eex[:, :, 24:48], eex[:, :, 0:24])
                    nc.scalar.copy(eex[:, :, 48:64], eex[:, :, 0:16])
                    nc.scalar.copy(eex[:, :, 64:65], eex[:, :, 0:1])

                # ---- multiply + aggregation for pair pb
                if 0 <= pb < NPAIR:
                    msg = workp.tile([P, NB2, 65], BF, tag="msg")
                    nc.vector.tensor_tensor(
                        out=msg[:], in0=gm_t[pb][:, 0:GE2],
                        in1=eex_t.pop(pb)[:],
                        op=mybir.AluOpType.mult,
                    )
                    msgv = msg[:].rearrange("p (w c) a -> p w c a", w=2)
                    pu2 = psum.tile([P, 2, 256], F32, tag="pu")
                    pu_t[pb] = pu2
                    for jj in range(2):
                        w = 2 * pb + jj
                        jS = w % 4
                        for c in range(C):
                            nc.tensor.matmul(
                                pu2[:, jj, 0:65],
                                lhsT=S4[pb][:, jS, c * P : (c + 1) * P],
                                rhs=msgv[:, jj, c, :],
                                start=(c == 0),
                                stop=(c == C - 1),
                            )

                # ---- output stage for pair pc
                if 0 <= pc < NPAIR:
                    pu_c = pu_t.pop(pc)
                    s_eps = small.tile([P, 2], F32, tag="s_eps")
                    nc.vector.tensor_scalar_add(s_eps[:], pu_c[:, :, 0], 1e-38)
                    rs = small.tile([P, 2], F32, tag="rs")
                    nc.vector.reciprocal(rs[:], s_eps[:])
                    if pc % 2 == 0:
                        yst = ystp.tile([P, 4, OUT_F], F32, tag="yst")
                    slot = 2 * (pc % 2)
                    ysb = yst[:, slot : slot + 2, :]
                    nc.vector.tensor_tensor(
                        out=ysb,
                        in0=pu_c[:, :, 1 : OUT_F + 1],
                        in1=rs[:].to_broadcast([P, 2, OUT_F]),
                        op=mybir.AluOpType.mult,
                    )
                    nc.vector.tensor_tensor(
                        out=ysb,
                        in0=ysb,
                        in1=b1_sb[:].rearrange("p (w f) -> p w f", w=2),
                        op=mybir.AluOpType.add,
                    )
                    if pc % 2 == 1 or pc == NPAIR - 1:
                        n = slot + 2
                        w1b = 2 * pc + 2 - n
                        nc.sync.dma_start(
                            y_out[:, w1b : w1b + n, :], yst[:, 0:n, :]
                        )
    return nc


# ------------------------------------------------------------- host helpers
def _run(nc, in_maps, label):
    profile = os.environ.get("GAT_PROFILE", "0") == "1"
    res = run_bass_kernel_spmd(
        nc, in_maps, core_ids=list(range(NC_CORES)), trace=profile
    )
    if profile:
        EXEC_TIMES_NS[label] = res.exec_time_ns
    return res.results


def _pack_nodes(dst):
    """Degree-balanced node->window permutation.  Returns perm [PADN] where
    perm[b*128 + p] = original node id placed at (bin b, position p);
    every bin's in-degree sum is <= C*128."""
    deg = np.bincount(dst, minlength=PADN).astype(np.int64)
    NBINS = NC_CORES * W_PER_CORE
    CAP = C * P
    order = np.argsort(-deg, kind="stable")
    arr = order.reshape(P, NBINS).copy()
    arr[1::2] = arr[1::2, ::-1]
    loads = deg[arr].sum(axis=0)
    nodes_by_bin = np.ascontiguousarray(arr.T)          # [NBINS, P]
    it = 0
    while loads.max() > CAP and it < 20000:
        b_hi = int(np.argmax(loads))
        b_lo = int(np.argmin(loads))
        excess = loads[b_hi] - CAP
        dh = deg[nodes_by_bin[b_hi]]
        dl = deg[nodes_by_bin[b_lo]]
        diff = dh[:, None] - dl[None, :]
        diff = np.where(diff > 0, diff, 10**9)
        i, j = np.unravel_index(np.argmin(np.abs(diff - excess)), diff.shape)
        d = dh[i] - dl[j]
        if d <= 0:
            break
        nodes_by_bin[b_hi, i], nodes_by_bin[b_lo, j] = (
            nodes_by_bin[b_lo, j],
            nodes_by_bin[b_hi, i],
        )
        loads[b_hi] -= d
        loads[b_lo] += d
        it += 1
    assert loads.max() <= CAP, f"window packing failed: max={loads.max()}"
    return nodes_by_bin.reshape(-1)


def _edge_slots(src, dst, packed_of):
    """Per-core edge->slot assignment in packed id space.  Returns
    (sidx, ddst, dloc): int64 [NC, W2, C*128] (pad = PADN), dloc f32."""
    psrc = packed_of[src]
    pdst = packed_of[dst]
    core = pdst // OWN
    win = (pdst % OWN) // P
    loc = pdst % P

    order = np.lexsort((win, core))
    s_src, s_core, s_win, s_loc = psrc[order], core[order], win[order], loc[order]
    group = s_core * W_PER_CORE + s_win
    cnt = np.bincount(group, minlength=NC_CORES * W_PER_CORE)
    gstart = np.zeros(NC_CORES * W_PER_CORE, dtype=np.int64)
    gstart[1:] = np.cumsum(cnt)[:-1]
    within = np.arange(len(order)) - gstart[group]
    assert within.max() < C * P

    sidx = np.full((NC_CORES, W2, C * P), PADN, dtype=np.int64)
    ddst = np.full((NC_CORES, W2, C * P), PADN, dtype=np.int64)
    dloc = np.full((NC_CORES, W2, C * P), -1.0, dtype=np.float32)
    sidx[s_core, s_win, within] = s_src
    ddst[s_core, s_win, within] = s_core * OWN + s_win * P + s_loc
    dloc[s_core, s_win, within] = s_loc.astype(np.float32)
    return sidx, ddst, dloc


def kernel(x, src, dst, W0, al0, ar0, b0, W1, al1, ar1, b1):
    _patch_tile()
    _install_ntff_hook()
    import ml_dtypes

    BFh = ml_dtypes.bfloat16
    F8h = ml_dtypes.float8_e4m3

    x = np.asarray(x, dtype=np.float32)
    src = np.asarray(src, dtype=np.int64)
    dst = np.asarray(dst, dtype=np.int64)
    W0 = np.asarray(W0, dtype=np.float32)
    al0 = np.asarray(al0, dtype=np.float32)
    ar0 = np.asarray(ar0, dtype=np.float32)
    b0 = np.asarray(b0, dtype=np.float32)
    W1 = np.asarray(W1, dtype=np.float32)
    al1 = np.asarray(al1, dtype=np.float32)
    ar1 = np.asarray(ar1, dtype=np.float32)
    b1 = np.asarray(b1, dtype=np.float32)

    HF = HEADS * HID

    # ---- weight prep
    vl0 = np.einsum("hd,hdk->hk", al0, W0.reshape(HEADS, HID, IN_F))
    vr0 = np.einsum("hd,hdk->hk", ar0, W0.reshape(HEADS, HID, IN_F))
    w0te = np.concatenate([W0.T, vl0.T, vr0.T], axis=1).astype(BFh)  # [256, 264]
    vl1 = al1 @ W1
    vr1 = ar1 @ W1
    w1te_orig = np.concatenate([W1.T, vl1.T, vr1.T], axis=1)         # [256, 66]
    # reorder features to the kernel's (d, h) layout
    jidx = np.arange(HF)
    feat_orig = (jidx % HEADS) * HID + jidx // HEADS
    w1te = w1te_orig[feat_orig].astype(BFh)
    b0T = np.ascontiguousarray(
        b0[feat_orig].reshape(2, P).T
    ).astype(np.float32)                                             # [128, 2]
    b1r = np.tile(b1[None, :], (P, 2)).astype(np.float32)

    # ---- node packing permutation
    perm = _pack_nodes(dst)                     # [PADN] packed -> orig
    packed_of = np.empty(PADN, dtype=np.int64)
    packed_of[perm] = np.arange(PADN)

    x_bf = x.astype(BFh)
    xg = np.zeros((PADN, IN_F), dtype=BFh)
    valid = perm < N_NODES
    xg[valid] = x_bf[perm[valid]]
    xT_pad = np.ascontiguousarray(xg.T)         # [256, PADN] packed order

    identb = np.eye(P, dtype=np.float32)

    # ---- K1: node tables
    nc1 = build_k1()
    in1 = [
        {
            "xT_own": np.ascontiguousarray(xT_pad[:, k * OWN : (k + 1) * OWN]),
            "w0te": w0te,
        }
        for k in range(NC_CORES)
    ]
    r1 = _run(nc1, in1, "k1")
    htab_h = np.concatenate(
        [r1[k]["htab_h"].transpose(1, 0, 2).reshape(OWN, IN_F) for k in range(NC_CORES)],
        axis=0,
    )
    htab_e = np.concatenate(
        [r1[k]["htab_e"].transpose(1, 0, 2).reshape(OWN, 8) for k in range(NC_CORES)],
        axis=0,
    )

    # ---- edge layout
    sidx, ddst, dloc = _edge_slots(src, dst, packed_of)

    htab_h_x = np.concatenate([htab_h, np.zeros((1, IN_F), dtype=BFh)], axis=0)
    htab_e_x = np.concatenate([htab_e, np.zeros((1, 8), dtype=BFh)], axis=0)

    # one-hot tiles: S[p, w, c*128+n] = (dloc == n), fp8
    def s_tiles(dl):
        d3 = dl.reshape(W2, C, P)
        oh = d3[:, :, :, None] == np.arange(P, dtype=np.float32)[None, None, None, :]
        return np.ascontiguousarray(
            oh.transpose(2, 0, 1, 3).reshape(P, W2, C * P).astype(F8h)
        )

    # ---- K2 inputs
    nc2 = build_k2(b0_zero=bool(np.all(b0 == 0.0)))
    in2 = []
    for k in range(NC_CORES):
        hg = htab_h_x[sidx[k]]                  # [W2, C*P, 256] bf16
        # -> he3 [P, pair, w2, c, a(65), h]; a=0 is all-ones (denominator),
        #    a=1+d holds hg[w, c*128+p, h*64+d]
        he_feat = (
            hg.reshape(NPAIR, 2, C, P, HEADS, HID)
            .transpose(3, 0, 1, 2, 5, 4)        # (p, pair, w2, c, d, h)
        )
        he3 = np.concatenate(
            [np.ones((P, NPAIR, 2, C, 1, HEADS), dtype=BFh), he_feat], axis=4
        ).reshape(P, NPAIR, HE2)
        meta = np.empty((W2, C * P, 8), dtype=BFh)
        meta[:, :, 0:4] = htab_e_x[sidx[k], 0:4]
        meta[:, :, 4:8] = htab_e_x[ddst[k], 4:8]
        meta3 = (
            meta.reshape(NPAIR, 2, C, P, 8)
            .transpose(3, 0, 1, 2, 4)           # (p, pair, w2, c, v)
            .reshape(P, NPAIR, MW2)
        )
        hm3 = np.concatenate([he3, meta3], axis=2)
        in2.append(
            {
                "hm3": np.ascontiguousarray(hm3),
                "S_in": s_tiles(dloc[k]),
                "b0T": b0T,
                "w1te": w1te,
                "identb": identb,
            }
        )
    r2 = _run(nc2, in2, "k2")
    gtab = np.concatenate(
        [
            r2[k]["g_out"][:, :W_PER_CORE, :].transpose(1, 0, 2).reshape(OWN, G)
            for k in range(NC_CORES)
        ],
        axis=0,
    )
    gtab_x = np.concatenate([gtab, np.zeros((1, G), dtype=gtab.dtype)], axis=0)

    # ---- K3 inputs
    nc3 = build_k3()
    in3 = []
    for k in range(NC_CORES):
        gg = gtab_x[sidx[k], :OUT_F]            # [W2, C*P, 64]
        g_feat = (
            gg.reshape(NPAIR, 2, C, P, OUT_F)
            .transpose(3, 0, 1, 2, 4)           # (p, pair, w2, c, a)
        )
        g3 = np.concatenate(
            [np.ones((P, NPAIR, 2, C, 1), dtype=BFh), g_feat], axis=4
        ).reshape(P, NPAIR, GE2)
        m1 = np.empty((W2, C * P, 2), dtype=BFh)
        m1[:, :, 0] = gtab_x[sidx[k], OUT_F]
        m1[:, :, 1] = gtab_x[ddst[k], OUT_F + 1]
        meta13 = (
            m1.reshape(NPAIR, 2, C, P, 2)
            .transpose(3, 0, 1, 4, 2)           # (p, pair, w2, v, c)
            .reshape(P, NPAIR, MW3)
        )
        gm3 = np.concatenate([g3, meta13], axis=2)
        in3.append(
            {
                "gm3": np.ascontiguousarray(gm3),
                "S_in": in2[k]["S_in"],
                "b1r": b1r,
            }
        )
    r3 = _run(nc3, in3, "k3")
    y_packed = np.concatenate(
        [
            r3[k]["y_out"][:, :W_PER_CORE, :].transpose(1, 0, 2).reshape(OWN, OUT_F)
            for k in range(NC_CORES)
        ],
        axis=0,
    )
    y_full = np.empty((PADN, OUT_F), dtype=np.float32)
    y_full[perm] = y_packed.astype(np.float32)
    return np.ascontiguousarray(y_full[:N_NODES])
